# revision 1
# baseline (speedup 1.0000x reference)
"""Trainium2 Bass kernel for nn_HandGNNEncoder (2-layer GCN on 21-node hand
graphs + mean pool), data-parallel over 8 NeuronCores.

Math restructure (exact):
  reference: h1 = relu(A @ (x @ W1) + b1); out = mean_t(A @ (h1 @ W2) + b2)
  mean-pool is linear, so with m[s] = column-mean of A (all > 0):
      out[g] = sum_s m[s] * h1[g,s,:] @ W2 + b2
  m[s] > 0 folds inside the relu:  m*relu(z) = relu(m*z).
  Stage 1 (PE): z[(s,f), g] = TW.T @ x'[g]   with TW[(s',c),(s,f)] =
      m[s]*A[s,s']*W1[c,f], bias row via a constant-1 input row, plus one
      extra column that relu's to the constant 1 (carries b2 in stage 2).
  Stage 2 (PE): out[d, g] = sum_k W2R_k.T @ relu_k  accumulated in PSUM.

Layouts are feature-major with graphs streaming as the matmul moving dim;
host does all transposes so every DMA is contiguous.
"""

import numpy as np

import concourse.bass as bass
import concourse.mybir as mybir
import concourse.tile as tile
from concourse import bass_utils

# ---- hardcoded problem constants ----
B, S, NNODE, CIN = 64, 512, 21, 2
D1, D2 = 64, 128
G = B * S                      # 32768 graphs
N_CORES = 8
G_CORE = G // N_CORES          # 4096 graphs per core
CHUNK = 512                    # graphs per pipeline chunk
N_CHUNKS = G_CORE // CHUNK
K1 = NNODE * CIN + 1           # 43 contraction rows (42 feats + ones row)
KT = 11                        # 1408 / 128 k-tiles for stage 2
M1 = KT * 128                  # 1408 = 1344 (s,f) cols + 1 bias col + 63 pad

EDGES = np.array(
    [[0, 1], [1, 2], [2, 3], [3, 4], [0, 5], [5, 6], [6, 7], [7, 8],
     [0, 9], [9, 10], [10, 11], [11, 12], [0, 13], [13, 14], [14, 15],
     [15, 16], [0, 17], [17, 18], [18, 19], [19, 20], [5, 9], [9, 13],
     [13, 17]], dtype=np.int64)


def fold_weights(W1, b1, W2, b2):
    """Fold adjacency, mean-pool and biases into two dense operands."""
    W1 = np.asarray(W1, np.float32)
    b1 = np.asarray(b1, np.float32)
    W2 = np.asarray(W2, np.float32)
    b2 = np.asarray(b2, np.float32)
    A = np.eye(NNODE, dtype=np.float32)
    A[EDGES[:, 1], EDGES[:, 0]] = 1.0
    deg = A.sum(axis=1)
    dis = 1.0 / np.sqrt(deg)
    a_norm = dis[:, None] * A * dis[None, :]          # [t, s] float32
    m = a_norm.mean(axis=0)                           # [21], all > 0

    # tw[(s',c), (s,f)] = m[s] * a_norm[s, s'] * W1[c, f]
    tw = np.zeros((K1, M1), np.float32)
    blk = np.einsum("s,st,cf->tcsf", m, a_norm, W1)   # [s'=t, c, s, f]
    tw[: NNODE * CIN, : NNODE * D1] = blk.reshape(NNODE * CIN, NNODE * D1)
    tw[K1 - 1, : NNODE * D1] = (m[:, None] * b1[None, :]).reshape(-1)
    tw[K1 - 1, NNODE * D1] = 1.0                      # relu's to constant 1

    w2full = np.zeros((M1, D2), np.float32)
    w2full[: NNODE * D1] = np.tile(W2, (NNODE, 1))
    w2full[NNODE * D1] = b2                           # rides the const-1 row
    # device tile is [128, KT*128] with pass-k slice [:, k*128:(k+1)*128]
    w2r = np.ascontiguousarray(
        w2full.reshape(KT, 128, D2).transpose(1, 0, 2).reshape(128, KT * D2))
    return tw, w2r


def build_bass(op_dt="float16"):
    f16 = getattr(mybir.dt, op_dt)
    f32 = mybir.dt.float32
    nc = bass.Bass("TRN2", target_bir_lowering=False, debug=False)
    xt_d = nc.dram_tensor("xt", [K1, G_CORE], f16, kind="ExternalInput").ap()
    tw_d = nc.dram_tensor("tw", [K1, M1], f16, kind="ExternalInput").ap()
    w2r_d = nc.dram_tensor("w2r", [128, KT * 128], f16,
                           kind="ExternalInput").ap()
    out_d = nc.dram_tensor("out", [D2, G_CORE], f32, kind="ExternalOutput").ap()

    relu = mybir.ActivationFunctionType.Relu
    copyf = mybir.ActivationFunctionType.Copy

    ACT_KS = {0, 2, 4, 6, 8}          # relu k-tiles on ScalarE; rest on DVE
    SKEW = 4                          # MM3'(u) issued after MM1(u+SKEW)
    NU = N_CHUNKS * KT                # 88 pipeline units

    with tile.TileContext(nc) as tc:
        with (
            tc.tile_pool(name="w", bufs=1) as wpool,
            tc.tile_pool(name="x", bufs=N_CHUNKS) as xpool,
            tc.tile_pool(name="ra", bufs=3) as rapool,
            tc.tile_pool(name="rv", bufs=3) as rvpool,
            tc.tile_pool(name="o", bufs=N_CHUNKS) as opool,
            tc.tile_pool(name="pa", bufs=3, space="PSUM") as papool,
            tc.tile_pool(name="pv", bufs=3, space="PSUM") as pvpool,
            tc.tile_pool(name="po", bufs=2, space="PSUM") as popool,
        ):
            # few big DMAs: descriptor-gen on the Sync sequencer costs
            # ~25ns/partition-row, so minimize rows and order by first use
            # partition-sliced loads spread transfers across HWDGE queues
            # (each queue is ~27 GiB/s) without adding descriptor rows
            tw_t = wpool.tile([K1, M1], f16, tag="tw")
            xt_t = xpool.tile([K1, G_CORE], f16, tag="xt")
            w2r_t = wpool.tile([128, KT * 128], f16, tag="w2r")
            for r0, r1 in ((0, 22), (22, 43)):
                nc.sync.dma_start(out=tw_t[r0:r1], in_=tw_d[r0:r1])
            for r0, r1 in ((0, 11), (11, 22), (22, 33), (33, 43)):
                nc.sync.dma_start(out=xt_t[r0:r1], in_=xt_d[r0:r1])
            for r0, r1 in ((0, 32), (32, 64), (64, 96), (96, 128)):
                nc.sync.dma_start(out=w2r_t[r0:r1], in_=w2r_d[r0:r1])
            tw_sb = [tw_t[:, k * 128:(k + 1) * 128] for k in range(KT)]
            w2r_sb = [w2r_t[:, k * 128:(k + 1) * 128] for k in range(KT)]
            xt_sb = [xt_t[:, ch * CHUNK:(ch + 1) * CHUNK]
                     for ch in range(N_CHUNKS)]

            out_ps = {}
            rts = {}

            def mm3(u):
                ch, k = divmod(u, KT)
                nc.tensor.matmul(
                    out_ps[ch],
                    lhsT=w2r_sb[k],
                    rhs=rts.pop(u),
                    start=(k == 0), stop=(k == KT - 1),
                    skip_group_check=True,
                )
                if k == KT - 1:
                    ot = opool.tile([D2, CHUNK], f32)
                    if ch % 2 == 0:
                        nc.scalar.activation(out=ot, in_=out_ps.pop(ch),
                                             func=copyf)
                    else:
                        nc.vector.tensor_copy(out=ot, in_=out_ps.pop(ch))
                    cs = ch * CHUNK
                    nc.sync.dma_start(out=out_d[:64, cs:cs + CHUNK],
                                      in_=ot[:64])
                    nc.sync.dma_start(out=out_d[64:, cs:cs + CHUNK],
                                      in_=ot[64:])

            for u in range(NU):
                ch, k = divmod(u, KT)
                if k == 0:
                    out_ps[ch] = popool.tile([D2, CHUNK], f32, tag="po",
                                             name=f"ops{ch}")
                on_act = k in ACT_KS
                pt = (papool if on_act else pvpool).tile(
                    [128, CHUNK], f32, tag="pa" if on_act else "pv")
                nc.tensor.matmul(
                    pt, lhsT=tw_sb[k], rhs=xt_sb[ch],
                    start=True, stop=True,
                )
                rt = (rapool if on_act else rvpool).tile(
                    [128, CHUNK], f16, tag="ra" if on_act else "rv")
                if on_act:
                    nc.scalar.activation(out=rt, in_=pt, func=relu)
                else:
                    nc.vector.tensor_scalar_max(out=rt, in0=pt, scalar1=0.0)
                rts[u] = rt
                if u >= SKEW:
                    mm3(u - SKEW)
            for u in range(NU - SKEW, NU):
                mm3(u)
    _rebalance_matmul_waits(nc)
    return nc


def _rebalance_matmul_waits(nc):
    """Walrus' TPB ISA structs accept only one sync-wait per instruction on
    the compute engines, but Tile can attach several (PE completion-order +
    cross-engine WAR + DMA). Keep one wait on the instruction and move the
    excess onto the immediately-preceding Ldweights (for matmuls) or onto
    freshly inserted same-engine NoOps — those execute just before on the
    same in-order queue, so waiting there is the same or stronger ordering."""
    import bass_rust
    import concourse.mybir as mybir

    exempt = {"InstEventSemaphore", "InstUnconditionalBranch",
              "InstCall", "InstISA", "InstNoOp"}
    nop_ctr = [0]
    for fn in nc.m.functions:
        for blk in fn.blocks:
            insts = list(blk.instructions)
            out = []
            pending_free_ldw = None
            for inst in insts:
                tn = type(inst).__name__
                if tn == "InstLdweights":
                    si = inst.sync_info
                    nw = len(si.on_wait) if si is not None else 0
                    if nw > 1:
                        for w in list(si.on_wait)[:-1]:
                            nop_ctr[0] += 1
                            nop = mybir.InstNoOp(
                                name=f"I-waitnop-{nop_ctr[0]}", ins=[],
                                outs=[])
                            nop.engine = inst.engine
                            nop.sync_info = bass_rust.SyncInfo(
                                on_wait=[w], on_update=[])
                            out.append(nop)
                        inst.sync_info = bass_rust.SyncInfo(
                            on_wait=list(si.on_wait)[-1:],
                            on_update=list(si.on_update))
                    elif nw == 0:
                        pending_free_ldw = inst
                    out.append(inst)
                    continue
                si = inst.sync_info
                nw = len(si.on_wait) if si is not None else 0
                if tn in exempt or nw <= 1:
                    out.append(inst)
                    if tn == "InstMatmult":
                        pending_free_ldw = None
                    continue
                waits = list(si.on_wait)
                moved, kept = waits[:-1], waits[-1:]
                if tn == "InstMatmult" and pending_free_ldw is not None \
                        and len(moved) == 1:
                    c = pending_free_ldw
                    csi = c.sync_info
                    c.sync_info = bass_rust.SyncInfo(
                        on_wait=moved,
                        on_update=list(csi.on_update) if csi else [])
                else:
                    for w in moved:
                        nop_ctr[0] += 1
                        nop = mybir.InstNoOp(
                            name=f"I-waitnop-{nop_ctr[0]}", ins=[], outs=[])
                        nop.engine = inst.engine
                        nop.sync_info = bass_rust.SyncInfo(
                            on_wait=[w], on_update=[])
                        out.append(nop)
                inst.sync_info = bass_rust.SyncInfo(
                    on_wait=kept, on_update=list(si.on_update))
                out.append(inst)
                if tn == "InstMatmult":
                    pending_free_ldw = None
            if len(out) != len(insts):
                blk.instructions = out


_NC_CACHE = None


def _get_nc():
    global _NC_CACHE
    if _NC_CACHE is None:
        _NC_CACHE = build_bass()
    return _NC_CACHE


def make_in_maps(hand_landmarks, W1, b1, W2, b2, np_dt=np.float16):
    tw, w2r = fold_weights(W1, b1, W2, b2)
    tw = tw.astype(np_dt)
    w2r = w2r.astype(np_dt)
    x = np.asarray(hand_landmarks, np.float32).reshape(G, NNODE * CIN)
    xt = np.empty((K1, G), np_dt)
    xt[: NNODE * CIN] = x.T
    xt[K1 - 1] = 1.0
    return [
        {
            "xt": np.ascontiguousarray(xt[:, i * G_CORE:(i + 1) * G_CORE]),
            "tw": tw,
            "w2r": w2r,
        }
        for i in range(N_CORES)
    ]


def gather_out(results):
    full = np.concatenate([results[i]["out"] for i in range(N_CORES)], axis=1)
    return np.ascontiguousarray(full.T).reshape(B, S, D2).astype(np.float32)


def run(in_maps, trace=False, **kw):
    res = bass_utils.run_bass_kernel_spmd(
        _get_nc(), in_maps, core_ids=list(range(N_CORES)), trace=trace, **kw)
    return res


def kernel(hand_landmarks, W1, b1, W2, b2):
    in_maps = make_in_maps(hand_landmarks, W1, b1, W2, b2)
    res = run(in_maps)
    return gather_out(res.results)



# revision 12
# speedup vs baseline: 1.4110x; 1.4110x over previous
"""Trainium2 Bass kernel for nn_HandGNNEncoder (2-layer GCN on 21-node hand
graphs + mean pool), data-parallel over 8 NeuronCores.

Math restructure (exact):
  reference: h1 = relu(A @ (x @ W1) + b1); out = mean_t(A @ (h1 @ W2) + b2)
  mean-pool is linear and commutes with W2: with m[s] = column-mean of A
  (all > 0) and m folded inside the relu (m*relu(z) = relu(m*z)):
      pooled[g,f] = sum_s relu(zm[g,s,f]),   zm = m[s]*(A(xW1)+b1)[s,f]
      out[g]      = pooled[g] @ W2 + b2
  Stage 1 (PE): zm[(s,f), g] = TW.T @ x'[g], TW[(s',c),(s,f)] =
      m[s]*A[s,s']*W1[c,f]; b1 rides a constant-1 input row.  11 k-tiles
      of 128 (s,f)-columns.
  Pooling: relu+accumulate fused into scalar_tensor_tensor ops
      (acc = max(psum,0)+acc) on DVE / Pool(gpsimd); chains initialized by
      ACT relu.  Two chains per chunk (even/odd k), merged to f16 on DVE.
  Stage 2 (PE): out[d,g] = W2STACK.T @ pooled128, one matmul per chunk,
      W2STACK[p,d] = W2[p%64,d] (partitions 0-63 = even-s sums, 64-127 =
      odd-s sums).  b2 is added on the host after gather.

PE work: 12 matmul columns/graph (vs 22 baseline); Ldweights amortized by
k-outer ordering (2 passes x 4 chunks) + a post-build pass that strips
redundant consecutive loads of the same stationary tile.
"""

import numpy as np

import concourse.bass as bass
import concourse.mybir as mybir
import concourse.tile as tile
from concourse import bass_utils

# ---- hardcoded problem constants ----
B, S, NNODE, CIN = 64, 512, 21, 2
D1, D2 = 64, 128
G = B * S                      # 32768 graphs
N_CORES = 8
G_CORE = G // N_CORES          # 4096 graphs per core
CHUNK = 512                    # graphs per chunk (one PSUM bank)
N_CHUNKS = G_CORE // CHUNK     # 8
K1 = NNODE * CIN + 1           # 43 contraction rows (42 feats + ones row)
KT = 11                        # k-tiles: 1408 / 128
M1 = KT * 128                  # 1344 (s,f) cols + 64 zero pad
N_PASSES = 2                   # chunks per pass = 4

EDGES = np.array(
    [[0, 1], [1, 2], [2, 3], [3, 4], [0, 5], [5, 6], [6, 7], [7, 8],
     [0, 9], [9, 10], [10, 11], [11, 12], [0, 13], [13, 14], [14, 15],
     [15, 16], [0, 17], [17, 18], [18, 19], [19, 20], [5, 9], [9, 13],
     [13, 17]], dtype=np.int64)


def fold_weights(W1, b1, W2):
    """Fold adjacency + mean-pool + b1 into TW; stack W2 for stage 2."""
    W1 = np.asarray(W1, np.float32)
    b1 = np.asarray(b1, np.float32)
    W2 = np.asarray(W2, np.float32)
    A = np.eye(NNODE, dtype=np.float32)
    A[EDGES[:, 1], EDGES[:, 0]] = 1.0
    deg = A.sum(axis=1)
    dis = 1.0 / np.sqrt(deg)
    a_norm = dis[:, None] * A * dis[None, :]          # [t, s]
    m = a_norm.mean(axis=0)                           # [21], all > 0

    tw = np.zeros((K1, M1), np.float32)
    blk = np.einsum("s,st,cf->tcsf", m, a_norm, W1)   # [s'=t, c, s, f]
    tw[: NNODE * CIN, : NNODE * D1] = blk.reshape(NNODE * CIN, NNODE * D1)
    tw[K1 - 1, : NNODE * D1] = (m[:, None] * b1[None, :]).reshape(-1)

    w2stack = np.concatenate([W2, W2], axis=0)        # [128, 128]
    return tw, w2stack


def build_bass(use_gpsimd=False, post=True):
    f16 = mybir.dt.float16
    f32 = mybir.dt.float32
    nc = bass.Bass("TRN2", target_bir_lowering=False, debug=False)
    xt_d = nc.dram_tensor("xt", [K1, G_CORE], f16, kind="ExternalInput").ap()
    tw_d = nc.dram_tensor("tw", [K1, M1], f16, kind="ExternalInput").ap()
    w2_d = nc.dram_tensor("w2", [D2, D2], f16, kind="ExternalInput").ap()
    out_d = nc.dram_tensor("out", [D2, G_CORE], f16, kind="ExternalOutput").ap()

    relu = mybir.ActivationFunctionType.Relu
    mx = mybir.AluOpType.max
    add = mybir.AluOpType.add
    CPP = N_CHUNKS // N_PASSES            # chunks per pass

    with tile.TileContext(nc) as tc:
        with (
            tc.tile_pool(name="w", bufs=1) as wpool,
            tc.tile_pool(name="a", bufs=1) as apool,
            tc.tile_pool(name="r", bufs=3) as rpool,
            tc.tile_pool(name="m", bufs=2) as mpool,
            tc.tile_pool(name="o", bufs=2) as opool,
            tc.tile_pool(name="p1", bufs=6, space="PSUM") as p1pool,
            tc.tile_pool(name="p2", bufs=2, space="PSUM") as p2pool,
        ):
            tw_t = wpool.tile([K1, M1], f16, tag="tw")
            xt_t = wpool.tile([K1, G_CORE], f16, tag="xt")
            w2_t = wpool.tile([D2, D2], f16, tag="w2")
            nc.sync.dma_start(out=tw_t, in_=tw_d)
            H = G_CORE // 2
            nc.sync.dma_start(out=xt_t[:, :H], in_=xt_d[:, :H])
            nc.sync.dma_start(out=xt_t[:, H:], in_=xt_d[:, H:])
            nc.sync.dma_start(out=w2_t, in_=w2_d)

            for p in range(N_PASSES):
                chs = range(p * CPP, (p + 1) * CPP)
                accA = {ch: apool.tile([128, CHUNK], f16, name=f"accA{ch}",
                                       tag=f"aA{ch % CPP}")
                        for ch in chs}
                accB = {ch: apool.tile([128, CHUNK], f16, name=f"accB{ch}",
                        tag=f"aB{ch % CPP}")
                        for ch in chs}
                # gpsimd cannot touch PSUM, so only DVE (stt) and ACT (relu)
                # drain stage-1 psums; gpsimd accumulates ACT's SBUF temps.
                ACT_KS = (3, 5, 7)          # ACT relu -> temp, gpsimd adds
                for k in range(KT):
                    lhs = tw_t[:, k * 128:(k + 1) * 128]
                    for ch in chs:
                        pt = p1pool.tile([128, CHUNK], f32, tag="p1")
                        nc.tensor.matmul(
                            pt, lhsT=lhs,
                            rhs=xt_t[:, ch * CHUNK:(ch + 1) * CHUNK],
                            start=True, stop=True)
                        if k == 0:
                            nc.scalar.activation(out=accA[ch], in_=pt,
                                                 func=relu)
                        elif k == 1:
                            nc.scalar.activation(out=accB[ch], in_=pt,
                                                 func=relu)
                        elif k in ACT_KS:
                            rt = rpool.tile([128, CHUNK], f16, tag="rt")
                            nc.scalar.activation(out=rt, in_=pt, func=relu)
                            eng = nc.gpsimd if use_gpsimd else nc.vector
                            eng.tensor_tensor(
                                out=accB[ch], in0=accB[ch], in1=rt, op=add)
                        else:
                            nc.vector.scalar_tensor_tensor(
                                out=accA[ch], in0=pt, scalar=0.0,
                                in1=accA[ch], op0=mx, op1=add)
                for ch in chs:
                    pooled = mpool.tile([128, CHUNK], f16, name=f"pool{ch}",
                                        tag="pool")
                    nc.vector.tensor_tensor(
                        out=pooled, in0=accA[ch], in1=accB[ch], op=add)
                    ops = p2pool.tile([D2, CHUNK], f32, tag="p2")
                    nc.tensor.matmul(ops, lhsT=w2_t, rhs=pooled,
                                     start=True, stop=True)
                    ot = opool.tile([D2, CHUNK], f16, name=f"ot{ch}",
                                    tag="ot")
                    nc.scalar.copy(out=ot, in_=ops)
                    cs = ch * CHUNK
                    nc.sync.dma_start(out=out_d[:, cs:cs + CHUNK], in_=ot)

    if post:
        _strip_redundant_ldweights(nc)
        _rebalance_matmul_waits(nc)
    return nc


def _ap_key(ap):
    return (ap.memref, ap.offset, tuple(tuple(d) for d in ap.ap))


def _strip_redundant_ldweights(nc):
    """Consecutive matmuls on the same stationary tile don't need to reload
    the PE array. Drop an InstLdweights when the previously-executed load on
    PE had an identical weights AP; carry any sync waits it held onto the
    next PE instruction."""
    import bass_rust

    for fn in nc.m.functions:
        for blk in fn.blocks:
            insts = list(blk.instructions)
            out = []
            last_key = None
            carry_waits = []
            for inst in insts:
                tn = type(inst).__name__
                if tn == "InstLdweights":
                    key = _ap_key(inst.ins[0])
                    if key == last_key:
                        si = inst.sync_info
                        if si is not None:
                            carry_waits.extend(si.on_wait)
                            assert not si.on_update, (
                                "won't drop ldweights holding sem updates")
                        continue  # drop the instruction
                    last_key = key
                elif tn == "InstMatmult" and carry_waits:
                    si = inst.sync_info
                    waits = list(si.on_wait) if si else []
                    ups = list(si.on_update) if si else []
                    inst.sync_info = bass_rust.SyncInfo(
                        on_wait=carry_waits + waits, on_update=ups)
                    carry_waits = []
                out.append(inst)
            assert not carry_waits
            if len(out) != len(insts):
                blk.instructions = out


def _rebalance_matmul_waits(nc):
    """Walrus' TPB ISA structs accept only one sync-wait per instruction on
    the compute engines, but Tile can attach several (PE completion-order +
    cross-engine WAR + DMA). Keep one wait on the instruction and move the
    excess onto the immediately-preceding Ldweights (for matmuls) or onto
    freshly inserted same-engine NoOps — those execute just before on the
    same in-order queue, so waiting there is the same or stronger ordering."""
    import bass_rust

    exempt = {"InstEventSemaphore", "InstUnconditionalBranch",
              "InstCall", "InstISA", "InstNoOp"}
    nop_ctr = [0]
    for fn in nc.m.functions:
        for blk in fn.blocks:
            insts = list(blk.instructions)
            out = []
            pending_free_ldw = None
            for inst in insts:
                tn = type(inst).__name__
                if tn == "InstLdweights":
                    si = inst.sync_info
                    nw = len(si.on_wait) if si is not None else 0
                    if nw > 1:
                        for w in list(si.on_wait)[:-1]:
                            nop_ctr[0] += 1
                            nop = mybir.InstNoOp(
                                name=f"I-waitnop-{nop_ctr[0]}", ins=[],
                                outs=[])
                            nop.engine = inst.engine
                            nop.sync_info = bass_rust.SyncInfo(
                                on_wait=[w], on_update=[])
                            out.append(nop)
                        inst.sync_info = bass_rust.SyncInfo(
                            on_wait=list(si.on_wait)[-1:],
                            on_update=list(si.on_update))
                    elif nw == 0:
                        pending_free_ldw = inst
                    out.append(inst)
                    continue
                si = inst.sync_info
                nw = len(si.on_wait) if si is not None else 0
                if tn in exempt or nw <= 1:
                    out.append(inst)
                    if tn == "InstMatmult":
                        pending_free_ldw = None
                    continue
                waits = list(si.on_wait)
                moved, kept = waits[:-1], waits[-1:]
                if tn == "InstMatmult" and pending_free_ldw is not None \
                        and len(moved) == 1:
                    c = pending_free_ldw
                    csi = c.sync_info
                    c.sync_info = bass_rust.SyncInfo(
                        on_wait=moved,
                        on_update=list(csi.on_update) if csi else [])
                else:
                    for w in moved:
                        nop_ctr[0] += 1
                        nop = mybir.InstNoOp(
                            name=f"I-waitnop-{nop_ctr[0]}", ins=[], outs=[])
                        nop.engine = inst.engine
                        nop.sync_info = bass_rust.SyncInfo(
                            on_wait=[w], on_update=[])
                        out.append(nop)
                inst.sync_info = bass_rust.SyncInfo(
                    on_wait=kept, on_update=list(si.on_update))
                out.append(inst)
                if tn == "InstMatmult":
                    pending_free_ldw = None
            if len(out) != len(insts):
                blk.instructions = out


_NC_CACHE = None


def _get_nc():
    global _NC_CACHE
    if _NC_CACHE is None:
        _NC_CACHE = build_bass()
    return _NC_CACHE


def make_in_maps(hand_landmarks, W1, b1, W2, b2, np_dt=np.float16):
    tw, w2stack = fold_weights(W1, b1, W2)
    tw = tw.astype(np_dt)
    w2stack = w2stack.astype(np_dt)
    x = np.asarray(hand_landmarks, np.float32).reshape(G, NNODE * CIN)
    xt = np.empty((K1, G), np_dt)
    xt[: NNODE * CIN] = x.T
    xt[K1 - 1] = 1.0
    return [
        {
            "xt": np.ascontiguousarray(xt[:, i * G_CORE:(i + 1) * G_CORE]),
            "tw": tw,
            "w2": w2stack,
        }
        for i in range(N_CORES)
    ]


def gather_out(results, b2):
    full = np.concatenate([results[i]["out"] for i in range(N_CORES)], axis=1)
    out = full.T.astype(np.float32) + np.asarray(b2, np.float32)[None, :]
    return np.ascontiguousarray(out).reshape(B, S, D2)


def run(in_maps, trace=False, **kw):
    res = bass_utils.run_bass_kernel_spmd(
        _get_nc(), in_maps, core_ids=list(range(N_CORES)), trace=trace, **kw)
    return res


def kernel(hand_landmarks, W1, b1, W2, b2):
    in_maps = make_in_maps(hand_landmarks, W1, b1, W2, b2)
    res = run(in_maps)
    return gather_out(res.results, b2)


# revision 17
# speedup vs baseline: 1.4338x; 1.0162x over previous
"""Trainium2 Bass kernel for nn_HandGNNEncoder (2-layer GCN on 21-node hand
graphs + mean pool), data-parallel over 8 NeuronCores.

Math restructure (exact):
  reference: h1 = relu(A @ (x @ W1) + b1); out = mean_t(A @ (h1 @ W2) + b2)
  mean-pool is linear and commutes with W2: with m[s] = column-mean of A
  (all > 0) and m folded inside the relu (m*relu(z) = relu(m*z)):
      pooled[g,f] = sum_s relu(zm[g,s,f]),   zm = m[s]*(A(xW1)+b1)[s,f]
      out[g]      = pooled[g] @ W2 + b2
  Stage 1 (PE): zm[(s,f), g] = TW.T @ x'[g], TW[(s',c),(s,f)] =
      m[s]*A[s,s']*W1[c,f]; b1 rides a constant-1 input row.  11 k-tiles
      of 128 (s,f)-columns.
  Pooling: relu+accumulate fused into scalar_tensor_tensor ops
      (acc = max(psum,0)+acc) on DVE / Pool(gpsimd); chains initialized by
      ACT relu.  Two chains per chunk (even/odd k), merged to f16 on DVE.
  Stage 2 (PE): out[d,g] = W2STACK.T @ pooled128, one matmul per chunk,
      W2STACK[p,d] = W2[p%64,d] (partitions 0-63 = even-s sums, 64-127 =
      odd-s sums).  b2 is added on the host after gather.

PE work: 12 matmul columns/graph (vs 22 baseline); Ldweights amortized by
k-outer ordering (2 passes x 4 chunks) + a post-build pass that strips
redundant consecutive loads of the same stationary tile.
"""

import numpy as np

import concourse.bass as bass
import concourse.mybir as mybir
import concourse.tile as tile
from concourse import bass_utils

# ---- hardcoded problem constants ----
B, S, NNODE, CIN = 64, 512, 21, 2
D1, D2 = 64, 128
G = B * S                      # 32768 graphs
N_CORES = 8
G_CORE = G // N_CORES          # 4096 graphs per core
CHUNK = 512                    # graphs per chunk (one PSUM bank)
N_CHUNKS = G_CORE // CHUNK     # 8
K1 = NNODE * CIN + 1           # 43 contraction rows (42 feats + ones row)
KT = 11                        # k-tiles: 1408 / 128
M1 = KT * 128                  # 1344 (s,f) cols + 64 zero pad
N_PASSES = 2                   # chunks per pass = 4

EDGES = np.array(
    [[0, 1], [1, 2], [2, 3], [3, 4], [0, 5], [5, 6], [6, 7], [7, 8],
     [0, 9], [9, 10], [10, 11], [11, 12], [0, 13], [13, 14], [14, 15],
     [15, 16], [0, 17], [17, 18], [18, 19], [19, 20], [5, 9], [9, 13],
     [13, 17]], dtype=np.int64)


def fold_weights(W1, b1, W2):
    """Fold adjacency + mean-pool + b1 into TW; stack W2 for stage 2."""
    W1 = np.asarray(W1, np.float32)
    b1 = np.asarray(b1, np.float32)
    W2 = np.asarray(W2, np.float32)
    A = np.eye(NNODE, dtype=np.float32)
    A[EDGES[:, 1], EDGES[:, 0]] = 1.0
    deg = A.sum(axis=1)
    dis = 1.0 / np.sqrt(deg)
    a_norm = dis[:, None] * A * dis[None, :]          # [t, s]
    m = a_norm.mean(axis=0)                           # [21], all > 0

    tw = np.zeros((K1, M1), np.float32)
    blk = np.einsum("s,st,cf->tcsf", m, a_norm, W1)   # [s'=t, c, s, f]
    tw[: NNODE * CIN, : NNODE * D1] = blk.reshape(NNODE * CIN, NNODE * D1)
    tw[K1 - 1, : NNODE * D1] = (m[:, None] * b1[None, :]).reshape(-1)

    w2stack = np.concatenate([W2, W2], axis=0)        # [128, 128]
    return tw, w2stack


def build_bass(use_gpsimd=True, post=True):
    f16 = mybir.dt.float16
    f32 = mybir.dt.float32
    nc = bass.Bass("TRN2", target_bir_lowering=False, debug=False)
    xt_d = nc.dram_tensor("xt", [K1, G_CORE], f16, kind="ExternalInput").ap()
    tw_d = nc.dram_tensor("tw", [K1, M1], f16, kind="ExternalInput").ap()
    w2_d = nc.dram_tensor("w2", [D2, D2], f16, kind="ExternalInput").ap()
    out_d = nc.dram_tensor("out", [D2, G_CORE], f16, kind="ExternalOutput").ap()

    relu = mybir.ActivationFunctionType.Relu
    mx = mybir.AluOpType.max
    add = mybir.AluOpType.add
    CPP = N_CHUNKS // N_PASSES            # chunks per pass

    with tile.TileContext(nc) as tc:
        with (
            tc.tile_pool(name="w", bufs=1) as wpool,
            tc.tile_pool(name="a", bufs=1) as apool,
            tc.tile_pool(name="r", bufs=3) as rpool,
            tc.tile_pool(name="m", bufs=2) as mpool,
            tc.tile_pool(name="o", bufs=2) as opool,
            tc.tile_pool(name="p1", bufs=6, space="PSUM") as p1pool,
            tc.tile_pool(name="p2", bufs=2, space="PSUM") as p2pool,
        ):
            tw_t = wpool.tile([K1, M1], f16, tag="tw")
            xt_t = wpool.tile([K1, G_CORE], f16, tag="xt")
            w2_t = wpool.tile([D2, D2], f16, tag="w2")
            # each dma_start transfers at ~22.5 GB/s on a single DMA engine;
            # parallelism comes from issuing many dma_starts (SP and ACT
            # sequencers both issue, ~0.6-1us per issue).  Slice xt per
            # chunk so pass-1 data lands early.
            for ch in range(N_CHUNKS):
                cs = ch * CHUNK
                nc.sync.dma_start(out=xt_t[:, cs:cs + CHUNK],
                                  in_=xt_d[:, cs:cs + CHUNK])
            nc.scalar.dma_start(out=tw_t[:, :256], in_=tw_d[:, :256])
            nc.scalar.dma_start(out=tw_t[:, 256:], in_=tw_d[:, 256:])
            nc.scalar.dma_start(out=w2_t, in_=w2_d)

            for p in range(N_PASSES):
                chs = range(p * CPP, (p + 1) * CPP)
                accA = {ch: apool.tile([128, CHUNK], f16, name=f"accA{ch}",
                                       tag=f"aA{ch % CPP}")
                        for ch in chs}
                # gpsimd crashes on in-place tensor_tensor; ping-pong accB
                accB = {ch: [apool.tile([128, CHUNK], f16,
                                        name=f"accB{ch}_{i}",
                                        tag=f"aB{ch % CPP}_{i}")
                             for i in range(2)]
                        for ch in chs}
                nB = {ch: 0 for ch in chs}
                # gpsimd cannot touch PSUM, so only DVE (stt) and ACT (relu)
                # drain stage-1 psums; gpsimd accumulates ACT's SBUF temps.
                ACT_KS = (3, 5, 7)          # ACT relu -> temp, gpsimd adds
                for k in range(KT):
                    lhs = tw_t[:, k * 128:(k + 1) * 128]
                    for ch in chs:
                        pt = p1pool.tile([128, CHUNK], f32, tag="p1")
                        nc.tensor.matmul(
                            pt, lhsT=lhs,
                            rhs=xt_t[:, ch * CHUNK:(ch + 1) * CHUNK],
                            start=True, stop=True)
                        if k == 0:
                            nc.scalar.activation(out=accA[ch], in_=pt,
                                                 func=relu)
                        elif k == 1:
                            nc.scalar.activation(out=accB[ch][0], in_=pt,
                                                 func=relu)
                        elif k in ACT_KS:
                            rt = rpool.tile([128, CHUNK], f16, tag="rt")
                            nc.scalar.activation(out=rt, in_=pt, func=relu)
                            i = nB[ch]
                            if use_gpsimd:
                                nc.gpsimd.tensor_tensor(
                                    out=accB[ch][(i + 1) % 2],
                                    in0=accB[ch][i % 2], in1=rt, op=add)
                                nB[ch] = i + 1
                            else:
                                nc.vector.tensor_tensor(
                                    out=accB[ch][0], in0=accB[ch][0],
                                    in1=rt, op=add)
                        else:
                            nc.vector.scalar_tensor_tensor(
                                out=accA[ch], in0=pt, scalar=0.0,
                                in1=accA[ch], op0=mx, op1=add)
                for ch in chs:
                    pooled = mpool.tile([128, CHUNK], f16, name=f"pool{ch}",
                                        tag="pool")
                    nc.vector.tensor_tensor(
                        out=pooled, in0=accA[ch],
                        in1=accB[ch][nB[ch] % 2], op=add)
                    ops = p2pool.tile([D2, CHUNK], f32, tag="p2")
                    nc.tensor.matmul(ops, lhsT=w2_t, rhs=pooled,
                                     start=True, stop=True)
                    ot = opool.tile([D2, CHUNK], f16, name=f"ot{ch}",
                                    tag="ot")
                    nc.scalar.copy(out=ot, in_=ops)
                    cs = ch * CHUNK
                    # split by rows: each dma_start runs on one DMA engine
                    nsplit = 4 if ch == N_CHUNKS - 1 else 2
                    rstep = D2 // nsplit
                    for r in range(0, D2, rstep):
                        nc.sync.dma_start(
                            out=out_d[r:r + rstep, cs:cs + CHUNK],
                            in_=ot[r:r + rstep])

    if post:
        _strip_redundant_ldweights(nc)
        _rebalance_matmul_waits(nc)
    return nc


def _ap_key(ap):
    return (ap.memref, ap.offset, tuple(tuple(d) for d in ap.ap))


def _strip_redundant_ldweights(nc):
    """Consecutive matmuls on the same stationary tile don't need to reload
    the PE array. Drop an InstLdweights when the previously-executed load on
    PE had an identical weights AP; carry any sync waits it held onto the
    next PE instruction."""
    import bass_rust

    for fn in nc.m.functions:
        for blk in fn.blocks:
            insts = list(blk.instructions)
            out = []
            last_key = None
            carry_waits = []
            for inst in insts:
                tn = type(inst).__name__
                if tn == "InstLdweights":
                    key = _ap_key(inst.ins[0])
                    if key == last_key:
                        si = inst.sync_info
                        if si is not None:
                            carry_waits.extend(si.on_wait)
                            assert not si.on_update, (
                                "won't drop ldweights holding sem updates")
                        continue  # drop the instruction
                    last_key = key
                elif tn == "InstMatmult" and carry_waits:
                    si = inst.sync_info
                    waits = list(si.on_wait) if si else []
                    ups = list(si.on_update) if si else []
                    inst.sync_info = bass_rust.SyncInfo(
                        on_wait=carry_waits + waits, on_update=ups)
                    carry_waits = []
                out.append(inst)
            assert not carry_waits
            if len(out) != len(insts):
                blk.instructions = out


def _rebalance_matmul_waits(nc):
    """Walrus' TPB ISA structs accept only one sync-wait per instruction on
    the compute engines, but Tile can attach several (PE completion-order +
    cross-engine WAR + DMA). Keep one wait on the instruction and move the
    excess onto the immediately-preceding Ldweights (for matmuls) or onto
    freshly inserted same-engine NoOps — those execute just before on the
    same in-order queue, so waiting there is the same or stronger ordering."""
    import bass_rust

    exempt = {"InstEventSemaphore", "InstUnconditionalBranch",
              "InstCall", "InstISA", "InstNoOp"}
    nop_ctr = [0]
    for fn in nc.m.functions:
        for blk in fn.blocks:
            insts = list(blk.instructions)
            out = []
            pending_free_ldw = None
            for inst in insts:
                tn = type(inst).__name__
                if tn == "InstLdweights":
                    si = inst.sync_info
                    nw = len(si.on_wait) if si is not None else 0
                    if nw > 1:
                        for w in list(si.on_wait)[:-1]:
                            nop_ctr[0] += 1
                            nop = mybir.InstNoOp(
                                name=f"I-waitnop-{nop_ctr[0]}", ins=[],
                                outs=[])
                            nop.engine = inst.engine
                            nop.sync_info = bass_rust.SyncInfo(
                                on_wait=[w], on_update=[])
                            out.append(nop)
                        inst.sync_info = bass_rust.SyncInfo(
                            on_wait=list(si.on_wait)[-1:],
                            on_update=list(si.on_update))
                    elif nw == 0:
                        pending_free_ldw = inst
                    out.append(inst)
                    continue
                si = inst.sync_info
                nw = len(si.on_wait) if si is not None else 0
                if tn in exempt or nw <= 1:
                    out.append(inst)
                    if tn == "InstMatmult":
                        pending_free_ldw = None
                    continue
                waits = list(si.on_wait)
                moved, kept = waits[:-1], waits[-1:]
                if tn == "InstMatmult" and pending_free_ldw is not None \
                        and len(moved) == 1:
                    c = pending_free_ldw
                    csi = c.sync_info
                    c.sync_info = bass_rust.SyncInfo(
                        on_wait=moved,
                        on_update=list(csi.on_update) if csi else [])
                else:
                    for w in moved:
                        nop_ctr[0] += 1
                        nop = mybir.InstNoOp(
                            name=f"I-waitnop-{nop_ctr[0]}", ins=[], outs=[])
                        nop.engine = inst.engine
                        nop.sync_info = bass_rust.SyncInfo(
                            on_wait=[w], on_update=[])
                        out.append(nop)
                inst.sync_info = bass_rust.SyncInfo(
                    on_wait=kept, on_update=list(si.on_update))
                out.append(inst)
                if tn == "InstMatmult":
                    pending_free_ldw = None
            if len(out) != len(insts):
                blk.instructions = out


_NC_CACHE = None


def _get_nc():
    global _NC_CACHE
    if _NC_CACHE is None:
        _NC_CACHE = build_bass()
    return _NC_CACHE


def make_in_maps(hand_landmarks, W1, b1, W2, b2, np_dt=np.float16):
    tw, w2stack = fold_weights(W1, b1, W2)
    tw = tw.astype(np_dt)
    w2stack = w2stack.astype(np_dt)
    x = np.asarray(hand_landmarks, np.float32).reshape(G, NNODE * CIN)
    xt = np.empty((K1, G), np_dt)
    xt[: NNODE * CIN] = x.T
    xt[K1 - 1] = 1.0
    return [
        {
            "xt": np.ascontiguousarray(xt[:, i * G_CORE:(i + 1) * G_CORE]),
            "tw": tw,
            "w2": w2stack,
        }
        for i in range(N_CORES)
    ]


def gather_out(results, b2):
    full = np.concatenate([results[i]["out"] for i in range(N_CORES)], axis=1)
    out = full.T.astype(np.float32) + np.asarray(b2, np.float32)[None, :]
    return np.ascontiguousarray(out).reshape(B, S, D2)


def run(in_maps, trace=False, **kw):
    res = bass_utils.run_bass_kernel_spmd(
        _get_nc(), in_maps, core_ids=list(range(N_CORES)), trace=trace, **kw)
    return res


def kernel(hand_landmarks, W1, b1, W2, b2):
    in_maps = make_in_maps(hand_landmarks, W1, b1, W2, b2)
    res = run(in_maps)
    return gather_out(res.results, b2)
